# revision 40
# baseline (speedup 1.0000x reference)
"""Trainium2 Bass kernel for EnhancedLocalAttention.

Reference semantics (B=4, L=4096, C=1024, H=16, D=64, W=512, STRIDE=256):
  qkv = x @ w_qkv + b_qkv -> q,k,v [B,H,L,D]
  15 overlapping windows of 512 tokens (stride 256); non-causal softmax
  attention inside each window; window outputs are CONCATENATED along the
  sequence (15*512=7680 rows), projected, truncated to the first L=4096 rows.

Consequences used here:
  - only windows 0..7 contribute to the final output
  - only tokens [0, 2304) of q/k/v are ever needed
  - output row p (p in [0,4096)) = window n=p//512, r=p%512,
    query token t=256n+r, attending to tokens [256n, 256n+512).

Sharding: 8 cores = (batch b in 0..3) x (window-group wg in 0..1).
  wg=0 -> windows 0..3, tokens [0,1280); wg=1 -> windows 4..7, tokens [1024,2304).
  Core (b,wg) computes output rows [2048*wg, 2048*wg+2048) of batch b.
  No cross-core communication.

Device pipeline (per core), all matmul operands bf16 (host pre-cast), PSUM
accumulation fp32, final output fp32:
  xT [1024,1280] (host pre-transposed, channels on partitions)
  QK^T [2048,1280]: matmul(lhsT=w_qkv cols, rhs=xT), bias fused into the
       PSUM->SBUF copy on ScalarE (per-partition bias AP).
  V [1280, 16*(64+1)] natural layout; 65th column per head = 1.0 so the
       PV matmul also emits the softmax denominators for free.
  per (window, head-pair): the two heads' K=64 scores^T matmuls go to PE row
       groups {0,1}/{2,3} (base partitions 0/64) and run concurrently in the
       systolic array; exp on ScalarE (no max subtraction; |scores| < ~3).
       Adjacent windows overlap 50%, so exp tiles are 768 queries wide and
       window w+1 writes only its new right-half into window w's chunk tiles
       (25% of scores^T matmuls and exp elements skipped).
    owT[65,512] = matmul(lhsT=V65, rhs=expT)
    oT rows = owT[:64] * broadcast(1/denom)   (approx-recip on DVE + GPSIMD
       partition_broadcast; denominators ~512 so 18-bit recip is exact enough)
  out[2048,1024] = matmul(lhsT=oT, rhs=w_out)
Scheduling: one software-pipelined loop; projections for window 0 are
interleaved as ACT-independent PE filler, scores^T of unit u+1 are emitted
before PV of unit u, and out-proj tiles of window w-1 drain between the
head-pairs of window w. This keeps the PE dense (HAM stays at 2.4 GHz).

b_qkv: q/k part applied on device; v part + b_out folded on host:
softmax rows sum to 1, so a v bias shifts each attention output by b_v and
the final correction is (b_v @ w_out + b_out) added to every output row.
"""

import sys

if "/opt/trn_rl_repo" not in sys.path:
    sys.path.insert(0, "/opt/trn_rl_repo")

import numpy as np

import concourse.bass as bass  # noqa: F401
from concourse import bacc, mybir
from concourse.tile import TileContext
from concourse.bass_utils import run_bass_kernel_spmd

B, L, C, H, D = 4, 4096, 1024, 16, 64
W, STRIDE = 512, 256
T = 1280          # tokens per core
NWIN = 4          # windows per core
P = 128
KT = C // P       # 8 contraction tiles of the channel dim
QKM = 2 * C // P  # 16 m-tiles of qk channels
F32 = mybir.dt.float32
F32R = mybir.dt.float32r
BF16 = mybir.dt.bfloat16

_PROG = None
_REUSE = True


def _build_program():
    nc = bacc.Bacc("TRN2", target_bir_lowering=False, debug=False, num_devices=8)

    xt = nc.dram_tensor("xt", [C, T], BF16, kind="ExternalInput").ap()
    wq = nc.dram_tensor("wq", [C, 3 * C], BF16, kind="ExternalInput").ap()
    wo = nc.dram_tensor("wo", [C, C], BF16, kind="ExternalInput").ap()
    bqk = nc.dram_tensor("bqk", [P, QKM], F32, kind="ExternalInput").ap()
    out = nc.dram_tensor("out", [NWIN * W, C], F32, kind="ExternalOutput").ap()

    wq_r = wq.rearrange("(ko p) n -> p ko n", p=P)
    wo_r = wo.rearrange("(ko p) n -> p ko n", p=P)

    NSL = ((0, 512), (512, 512), (1024, 256))  # token n-tiles of T=1280

    with TileContext(nc) as tc:
        with tc.tile_pool(name="persist", bufs=1) as persist:
            QKS = persist.tile([P, QKM, T], BF16)        # 40 KB/part
            BQK = persist.tile([P, QKM], F32)
            nc.sync.dma_start(out=BQK[:], in_=bqk)
            NVT = T // P                                  # 10 token tiles
            VE = persist.tile([P, NVT, H, D + 1], BF16)    # 40.6 KB/part
            nc.vector.memset(VE[:, :, :, D : D + 1].bitcast(mybir.dt.uint16), 0x3F80)

            # ===== merged pipeline: projections + attention + out-proj =====
            with (
                tc.tile_pool(name="xtp", bufs=1) as xtp,
                tc.tile_pool(name="wcolq", bufs=3) as wcolq,
                tc.tile_pool(name="wcolv", bufs=1) as wcolv,
                tc.tile_pool(name="wop", bufs=1) as wop,
                tc.tile_pool(name="exp", bufs=18) as expool,
                tc.tile_pool(name="norm", bufs=2) as norm,
                tc.tile_pool(name="otp", bufs=2) as otp,
                tc.tile_pool(name="ostage", bufs=2) as ostage,
                tc.tile_pool(name="ps_st", bufs=2, space="PSUM") as ps_st,
                tc.tile_pool(name="ps_ow", bufs=2, space="PSUM") as ps_ow,
                tc.tile_pool(name="ps_out", bufs=2, space="PSUM") as ps_out,
            ):
                # spread the latency-critical initial loads across three DMA
                # queues (sync/scalar HWDGE + gpsimd SWDGE) so they overlap
                XT = xtp.tile([P, KT, T], BF16)            # 20 KB/part
                xt_r = xt.rearrange("(ko p) t -> p ko t", p=P)
                qs = [nc.sync, nc.scalar, nc.gpsimd]
                for k in range(KT):
                    qs[k % 3].dma_start(out=XT[:, k, :], in_=xt_r[:, k, :])
                WCV = wcolv.tile([P, KT, C], BF16)         # v weights, 16 KB
                for k in range(KT):
                    nc.gpsimd.dma_start(
                        out=WCV[:, k, :], in_=wq_r[:, k, 2 * C : 3 * C]
                    )
                WO = wop.tile([P, KT, C], BF16)            # 16 KB/part

                def emit_projpair(j):
                    # qk^T projection for m-tiles j (q) and 8+j (k): exactly
                    # the QKS tiles head pair j of every window needs.
                    for mi in (j, KT + j):
                        wct = wcolq.tile([P, KT, P], BF16, tag="wq", name=f"wq_{mi}")
                        nc.sync.dma_start(
                            out=wct[:], in_=wq_r[:, :, mi * P : (mi + 1) * P]
                        )
                        for ni, (n0, ln) in enumerate(NSL):
                            pp = ps_st.tile(
                                [P, 2, W], F32, tag="st", name=f"pp_{mi}_{ni}"
                            )
                            for k in range(KT):
                                nc.tensor.matmul(
                                    pp[:, 0, :ln],
                                    wct[:, k, :],
                                    XT[:, k, n0 : n0 + ln],
                                    start=(k == 0),
                                    stop=(k == KT - 1),
                                )
                            nc.scalar.activation(
                                QKS[:, mi, n0 : n0 + ln],
                                pp[:, 0, :ln],
                                mybir.ActivationFunctionType.Identity,
                                bias=BQK[:, mi : mi + 1],
                                scale=1.0,
                            )

                def emit_vproj(mt):
                    # v projection for one token tile (both channel halves)
                    for nh in range(2):
                        pv = ps_st.tile([P, 2, W], F32, tag="st", name=f"pv_{mt}_{nh}")
                        for k in range(KT):
                            nc.tensor.matmul(
                                pv[:, 0, :],
                                XT[:, k, mt * P : (mt + 1) * P],
                                WCV[:, k, nh * 512 : (nh + 1) * 512],
                                start=(k == 0),
                                stop=(k == KT - 1),
                            )
                        nc.vector.tensor_copy(
                            out=VE[:, mt, nh * 8 : (nh + 1) * 8, 0:D],
                            in_=pv[:, 0, :].rearrange("p (h d) -> p h d", d=D),
                        )

                EXPF = mybir.ActivationFunctionType.Exp
                SCL = float(1.0 / np.sqrt(D))

                WQ3 = W + 256  # EC tiles carry 256 extra query cols

                def emit_st(j, w, prev, u):
                    # scores^T for head pair (2j, 2j+1) of window w. The two
                    # heads' K=64 matmuls go to PE row groups {0,1} and {2,3}
                    # (base partitions 0 / 64) and run concurrently.
                    # Windows overlap 50%: exp(scores) for k-chunks {0,1} at
                    # queries [0,256) equals window w-1's chunks {2,3} at
                    # queries [256,512). EC tiles are 768 wide; for w>0 only
                    # the new right-half of chunks {0,1} is computed and its
                    # exp is written into the PREVIOUS window's chunk tiles at
                    # cols [512:768), so PV reads one contiguous [128,512] AP.
                    q0 = w * STRIDE
                    mq = j
                    mk = KT + j
                    ch = {}
                    if prev is not None:
                        for a in range(2):
                            s1 = ps_st.tile(
                                [P, 2, W], F32, tag="st", name=f"s1_{u}_{a}"
                            )
                            for hd in range(2):
                                po = hd * D
                                nc.tensor.matmul(
                                    s1[:, hd, 0:256],
                                    QKS[po : po + D, mk, q0 + a * P : q0 + (a + 1) * P],
                                    QKS[po : po + D, mq, q0 + 256 : q0 + W],
                                    start=True,
                                    stop=True,
                                )
                            nc.scalar.activation(
                                prev[2 + a][0][:, :, W:WQ3],
                                s1[:, :, 0:256],
                                EXPF,
                                scale=SCL,
                            )
                        ch[0] = (prev[2][0], 256)
                        ch[1] = (prev[3][0], 256)
                        rng = []
                    else:
                        rng = [0, 1]
                    for c in rng:
                        st = ps_st.tile([P, 2, W], F32, tag="st", name=f"st_{u}_{c}")
                        for hd in range(2):
                            po = hd * D
                            nc.tensor.matmul(
                                st[:, hd, :],
                                QKS[po : po + D, mk, q0 + c * P : q0 + (c + 1) * P],
                                QKS[po : po + D, mq, q0 : q0 + W],
                                start=True,
                                stop=True,
                            )
                        ec = expool.tile([P, 2, WQ3], BF16, tag="ec", name=f"ec_{u}_{c}")
                        nc.scalar.activation(ec[:, :, 0:W], st[:], EXPF, scale=SCL)
                        ch[c] = (ec, 0)
                    return ch

                def emit_st2(j, w, ch, u):
                    # chunks 2,3 of unit u (always computed in full)
                    q0 = w * STRIDE
                    mq = j
                    mk = KT + j
                    for c in (2, 3):
                        st = ps_st.tile([P, 2, W], F32, tag="st", name=f"st_{u}_{c}")
                        for hd in range(2):
                            po = hd * D
                            nc.tensor.matmul(
                                st[:, hd, :],
                                QKS[po : po + D, mk, q0 + c * P : q0 + (c + 1) * P],
                                QKS[po : po + D, mq, q0 : q0 + W],
                                start=True,
                                stop=True,
                            )
                        ec = expool.tile([P, 2, WQ3], BF16, tag="ec", name=f"ec_{u}_{c}")
                        nc.scalar.activation(ec[:, :, 0:W], st[:], EXPF, scale=SCL)
                        ch[c] = (ec, 0)

                def emit_pv1(j, w, hd, ch, u):
                    # PV accumulation over chunks 0,1 (their exps finish first)
                    h = 2 * j + hd
                    ow = ps_ow.tile([P, W], F32, tag="ow", name=f"ow_{u}_{hd}")
                    for c in (0, 1):
                        ec, off = ch[c]
                        nc.tensor.matmul(
                            ow[0 : D + 1, :],
                            VE[:, 2 * w + c, h, :],
                            ec[:, hd, off : off + W],
                            start=(c == 0),
                            stop=False,
                        )
                    return ow

                def emit_pv2(j, w, hd, ch, ow, OT, u):
                    h = 2 * j + hd
                    po = hd * D
                    for c in (2, 3):
                        ec, off = ch[c]
                        nc.tensor.matmul(
                            ow[0 : D + 1, :],
                            VE[:, 2 * w + c, h, :],
                            ec[:, hd, off : off + W],
                            start=False,
                            stop=(c == 3),
                        )
                    dcp = norm.tile([1, W], F32, tag="dcp", name=f"dcp_{u}_{hd}")
                    nc.vector.tensor_copy(out=dcp, in_=ow[D : D + 1, :])
                    rd = norm.tile([1, W], F32, tag="rd", name=f"rd_{u}_{hd}")
                    nc.vector.reciprocal_approx_fast(out=rd, in_=dcp)
                    rdb = norm.tile([D, W], F32, tag="rdb", name=f"rdb_{u}_{hd}")
                    nc.gpsimd.partition_broadcast(rdb, rd)
                    nc.vector.tensor_tensor(
                        OT[po : po + D, j, :],
                        ow[0:D, :],
                        rdb,
                        mybir.AluOpType.mult,
                    )

                def emit_outproj(w, OT, no, m, tag):
                    pop = ps_out.tile([P, 512], F32, tag="po", name=f"po_{tag}")
                    for k in range(KT):
                        nc.tensor.matmul(
                            pop,
                            OT[:, k, m * P : (m + 1) * P],
                            WO[:, k, no * 512 : (no + 1) * 512],
                            start=(k == 0),
                            stop=(k == KT - 1),
                        )
                    ost = ostage.tile([P, 512], F32, tag="ost", name=f"ost_{tag}")
                    nc.vector.tensor_copy(out=ost, in_=pop)
                    nc.sync.dma_start(
                        out=out[
                            w * W + m * P : w * W + (m + 1) * P,
                            no * 512 : (no + 1) * 512,
                        ],
                        in_=ost,
                    )

                # software pipeline over (window w, head-pair j) units.
                # ST(u+1) is emitted before PV(u) so the PE has work while
                # ScalarE runs exp(u); chunk tiles of window w-1 stay alive a
                # full window for the overlap reuse; out-proj tiles of window
                # w-1 are interleaved between pairs of window w as
                # ACT-independent PE filler.
                units = [(w, j) for w in range(NWIN) for j in range(KT)]
                OTs = {w: None for w in range(NWIN)}
                exws = {}
                pending = []
                # prologue: v tiles for window 0 + first two qk proj pairs;
                # the rest become ACT-independent PE filler inside the loop
                for mt in range(4):
                    emit_vproj(mt)
                emit_projpair(0)
                emit_projpair(1)
                # w_out is first needed by out-proj jobs much later; load it
                # after the latency-critical weights above
                nc.sync.dma_start(out=WO[:], in_=wo_r)
                fillers = [(lambda jj: lambda: emit_projpair(jj))(jj) for jj in range(2, KT)]
                fillers += [(lambda mm: lambda: emit_vproj(mm))(mm) for mm in range(4, NVT)]
                OTs[0] = otp.tile([P, KT, W], BF16, tag="ot", name="ot_0")
                exws[(0, 0)] = emit_st(0, 0, None, 0)
                emit_st2(0, 0, exws[(0, 0)], 0)
                for u in range(len(units)):
                    w, j = units[u]
                    if fillers:
                        fillers.pop(0)()
                    nxt = None
                    if u + 1 < len(units):
                        wn, jn = units[u + 1]
                        if OTs[wn] is None:
                            OTs[wn] = otp.tile(
                                [P, KT, W], BF16, tag="ot", name=f"ot_{wn}"
                            )
                        prev = exws.get((wn - 1, jn)) if _REUSE else None
                        exws[(wn, jn)] = emit_st(jn, wn, prev, u + 1)
                        nxt = (wn, jn)
                    ch = exws[(w, j)]
                    ow0 = emit_pv1(j, w, 0, ch, u)
                    ow1 = emit_pv1(j, w, 1, ch, u)
                    if nxt is not None:
                        emit_st2(nxt[1], nxt[0], exws[nxt], u + 1)
                    if j != KT - 1:
                        for pi in range(min(2, len(pending))):
                            pending.pop(0)(f"{w}_{j}_{pi}")
                    emit_pv2(j, w, 0, ch, ow0, OTs[w], u)
                    emit_pv2(j, w, 1, ch, ow1, OTs[w], u)
                    exws.pop((w - 1, j), None)
                    if j == KT - 1:
                        pending.extend(
                            (lambda ww, no, m: lambda tag: emit_outproj(
                                ww, OTs[ww], no, m, f"{tag}_{no}_{m}"
                            ))(w, no, m)
                            for no in range(2)
                            for m in range(4)
                        )
                for jf, job in enumerate(pending):
                    job(f"f_{jf}")

    nc.compile()
    return nc


def _get_program():
    global _PROG
    if _PROG is None:
        _PROG = _build_program()
    return _PROG


def _make_in_maps(x, w_qkv, b_qkv, w_out):
    import ml_dtypes
    bf16 = ml_dtypes.bfloat16
    x = np.asarray(x, dtype=np.float32)
    w_qkv = np.ascontiguousarray(np.asarray(w_qkv, dtype=np.float32).astype(bf16))
    wo = np.ascontiguousarray(np.asarray(w_out, dtype=np.float32).astype(bf16))
    bqk = np.ascontiguousarray(
        np.asarray(b_qkv[: 2 * C], dtype=np.float32).reshape(QKM, P).T
    )
    in_maps = []
    for core in range(8):
        b, wg = divmod(core, 2)
        t0 = 1024 * wg
        xT = np.ascontiguousarray(x[b, t0 : t0 + T, :].T.astype(bf16))
        in_maps.append({"xt": xT, "wq": w_qkv, "wo": wo, "bqk": bqk})
    return in_maps


def _run(x, w_qkv, b_qkv, w_out, b_out, trace=False, **spmd_kwargs):
    nc = _get_program()
    in_maps = _make_in_maps(x, w_qkv, b_qkv, w_out)
    res = run_bass_kernel_spmd(nc, in_maps, list(range(8)), trace=trace, **spmd_kwargs)
    corr = (
        np.asarray(b_qkv[2 * C :], dtype=np.float32) @ np.asarray(w_out, dtype=np.float32)
        + np.asarray(b_out, dtype=np.float32)
    ).astype(np.float32)
    outp = np.empty((B, L, C), dtype=np.float32)
    for core in range(8):
        b, wg = divmod(core, 2)
        outp[b, wg * 2048 : (wg + 1) * 2048, :] = res.results[core]["out"] + corr
    return outp, res


def kernel(x, w_qkv, b_qkv, w_out, b_out):
    outp, _ = _run(x, w_qkv, b_qkv, w_out, b_out, trace=False)
    return outp
